# revision 11
# baseline (speedup 1.0000x reference)
"""GroupGMM Trainium2 kernel (fp8 DoubleRow, resident weights).

Computes, for B=8192 samples with soft group-mixture weights over G=32 groups:
    logits = einsum("bi,gio,bg->bo", x, W_pi, g) + g @ b_pi        [B, 16]
    loc    = einsum(... W_mu ...)   + g @ b_mu                     [B, 512]
    scale  = softplus(einsum(... W_sigma ...) + g @ b_sigma)+1e-7  [B, 512]
    out    = concat([logits, loc, scale], -1)                      [B, 1040]

Strategy: data-parallel over batch across 8 NeuronCores (1024 rows each).
The group einsum folds into one matmul with contraction K = G*I = 16384 via
z[b,(g,i)] = g[b,g] * x[b,i], run in fp8e4 (e4m3) with the PE's DoubleRow
perf mode (two 128-row K-tiles per pass at 0.5 cycles/row; measured overall
rel err ~1e-2 vs the 2e-2 gate). At fp8-DR speed the kernel sits on the
cost model's ridge: PE needs ~217ns per K-pair-chunk, the single 360GB/s
DMA_ENGINES resource needs ~100us for the minimal HBM traffic, and every
DMA dispatch costs ~0.7us of its sequencer (HWDGE) or ~1us of the Pool
engine (SWDGE). Layout of the work:
  - The 17MB fp8 weight tensor is RESIDENT, loaded once during sweep 1 as
    64 per-pair [128, 2, 1040] tiles on the sync HWDGE queue (a DoubleRow
    matmul's moving operand must come from a tile whose partition stride
    equals the slice free size — slicing a big 4D tile silently reads the
    wrong addresses in BIRSim — and re-streaming per sweep would triple W
    traffic). Sweep 1 is therefore W-stream-paced (~55us); sweeps 2-3 are
    PE-paced.
  - Gates load as per-sweep [128, 8-groups, mw] column-slice tiles on the
    gpsimd SWDGE queue (keeps the HWDGE/W stream clean); bias as one
    [128, chunks, 1040] tile per sweep and x^T as one resident
    [128, 4, BLOC] tile on the sync queue, sliced small at startup so the
    first matmul issues at ~3us. Output writes ride the ACT HWDGE queue so
    a compute-dependent store can never head-of-line-block a load.
  - z tiles are built per K-pair: ONE DVE multiply [128, 2, mw] bf16 (x^T
    pair slice x gate row broadcast by a stride-0 AP — 2x DVE mode), then
    cast bf16->fp8 on a rotating engine (8 ACT / 6 Pool / 2 DVE per 16)
    because a DVE multiply cannot write fp8 at 2x and no single helper
    engine can match the PE pace. GPSIMD cannot touch PSUM, so all drain
    adds stay on DVE.
Per 128-sample chunk each pair issues 3 DR matmuls (mu 512, sigma 512,
pi 16 cols; lhsT [128, 2, 128]) — the shapes concourse's tile_matmul
emits. PSUM: 3 chunks x (mu+sg) + 2 pi banks = 8, so the batch runs in 3
sweeps ([0..2],[3..5],[6..7]). Sweep boundaries pipeline: the last 12
pairs of a sweep interleave generation of the next sweep's first 6 z
tiles, and those 6 pairs are emitted chunk-major so the PE restarts on
chunk 0 as soon as its two drain adds retire. The bias term g @ b_cat is
precomputed on the host in f32 and added at drain time on DVE; sigma gets
softplus via ACT Exp+Ln.
"""

import numpy as np
import ml_dtypes

import concourse.bass as bass
import concourse.tile as tile
from concourse import bacc, mybir
from concourse.bass_utils import run_bass_kernel_spmd

B, I, G, C, D = 8192, 512, 32, 16, 32
CD = C * D                      # 512
OUT_W = C + 2 * CD              # 1040
NCORES = 8
BLOC = B // NCORES              # 1024
KTOT = G * I                    # 16384
NKT = KTOT // 128               # 128 K-tiles
NPAIR = NKT // 2                # 64 DoubleRow K-tile pairs
NMC = BLOC // 128               # 8 sample chunks per core
SWEEPS = [[0, 1, 2], [3, 4, 5], [6, 7]]
NCARRY = 6                      # pairs pre-generated across a sweep boundary

BF16 = mybir.dt.bfloat16
F32 = mybir.dt.float32
FP8 = mybir.dt.float8e4
DR = mybir.MatmulPerfMode.DoubleRow

# Rotating engine for the bf16->fp8 z cast (by pair index mod 16).
_CAST_DVE = {5, 13}
_CAST_POOL = {1, 3, 7, 9, 11, 15}

_cache: dict = {}


def _build_program():
    if "nc" in _cache:
        return _cache["nc"]
    from contextlib import ExitStack

    nc = bacc.Bacc("TRN2", target_bir_lowering=False, debug=False)

    xt_d = nc.dram_tensor("xt", [128, 4, BLOC], BF16, kind="ExternalInput")
    gb_d = nc.dram_tensor("gb", [128, G, BLOC], BF16, kind="ExternalInput")
    w_d = nc.dram_tensor("w", [NPAIR, 128, 2, OUT_W], FP8, kind="ExternalInput")
    bias_d = nc.dram_tensor("bias", [128, NMC, OUT_W], F32,
                            kind="ExternalInput")
    out_d = nc.dram_tensor("out", [128, NMC, OUT_W], F32,
                           kind="ExternalOutput")

    with tile.TileContext(nc) as tc, ExitStack() as ctx:
        res = ctx.enter_context(tc.tile_pool(name="res", bufs=1))
        gp = ctx.enter_context(tc.tile_pool(name="gp", bufs=3))
        zbp = ctx.enter_context(tc.tile_pool(name="zbp", bufs=5))
        zp = ctx.enter_context(tc.tile_pool(name="zp", bufs=8))
        op = ctx.enter_context(tc.tile_pool(name="op", bufs=3))
        bp = ctx.enter_context(tc.tile_pool(name="bp", bufs=1))
        pp = ctx.enter_context(tc.tile_pool(name="pp", bufs=1, space="PSUM"))

        # ---- startup loads ----
        # Small first slices so pair 0's z-mul starts ~1.5us in: x^T blocks
        # 0-1 for sweep-1 columns only, gates for groups 0-1.
        xt4 = res.tile([128, 4, BLOC], BF16, name="xt4", tag="xt4")
        nc.sync.dma_start(xt4[:, 0:2, 0:384], xt_d[:, 0:2, 0:384])

        gbt: dict = {}

        def issue_gb(s, t, split=False):
            mcs = SWEEPS[s]
            m0 = mcs[0] * 128
            mw = len(mcs) * 128
            tl = gp.tile([128, 8, mw], BF16, name=f"gb{s}_{t}", tag="gbs")
            if split:
                nc.gpsimd.dma_start(tl[:, 0:2, :],
                                    gb_d[:, t * 8:t * 8 + 2, m0:m0 + mw])
                nc.gpsimd.dma_start(tl[:, 2:8, :],
                                    gb_d[:, t * 8 + 2:(t + 1) * 8, m0:m0 + mw])
            else:
                nc.gpsimd.dma_start(tl[:],
                                    gb_d[:, t * 8:(t + 1) * 8, m0:m0 + mw])
            gbt[(s, t)] = tl

        issue_gb(0, 0, split=True)

        # Resident W: 64 individual [128, 2, OUT_W] tiles on the sync queue.
        wres = [res.tile([128, 2, OUT_W], FP8, name=f"w{pr}", tag=f"w{pr}")
                for pr in range(NPAIR)]
        nc.sync.dma_start(wres[0][:], w_d[0])
        nc.sync.dma_start(wres[1][:], w_d[1])
        nc.sync.dma_start(xt4[:, 0:2, 384:], xt_d[:, 0:2, 384:])
        for pr in range(2, 8):
            nc.sync.dma_start(wres[pr][:], w_d[pr])
        nc.sync.dma_start(xt4[:, 2:4, :], xt_d[:, 2:4, :])
        issue_gb(0, 1)

        carry_z: dict = {}

        def gen_z(s, pr, mcs):
            gi = pr // 2
            xb0 = (pr % 2) * 2
            m0 = mcs[0] * 128
            mw = len(mcs) * 128
            gsl = gbt[(s, gi // 8)][:, gi % 8, :].unsqueeze(1).broadcast_to(
                [128, 2, mw])
            xsl = xt4[:, xb0:xb0 + 2, m0:m0 + mw]
            zt = zp.tile([128, 2, mw], FP8, name=f"zt{s}_{pr}", tag="zt")
            m = pr % 16
            if m in _CAST_DVE or (s == 0 and pr < 4):
                # Direct fp8-out multiply (1x DVE): used for the DVE share
                # and at startup, where an ACT cast would pull the first
                # matmul behind a 1.3us act-table load.
                nc.vector.tensor_mul(zt[:], xsl, gsl)
                return zt
            zb = zbp.tile([128, 2, mw], BF16, name=f"zb{s}_{pr}", tag="zb")
            nc.vector.tensor_mul(zb[:], xsl, gsl)
            if m in _CAST_POOL:
                nc.gpsimd.tensor_copy(zt[:], zb[:])
            else:
                nc.scalar.activation(zt[:], zb[:],
                                     mybir.ActivationFunctionType.Copy)
            return zt

        def pair_matmuls(zt, pr, mcs, pmu, psg, ppi, chunks=None):
            first = pr == 0
            last = pr == NPAIR - 1
            for j, mc in (chunks if chunks is not None else enumerate(mcs)):
                lhs = zt[:, :, j * 128:(j + 1) * 128]
                if last:
                    # sigma first so its drain chain starts earliest
                    nc.tensor.matmul(psg[mc][:], lhs,
                                     wres[pr][:, :, C + CD:],
                                     start=False, stop=True, perf_mode=DR)
                    nc.tensor.matmul(pmu[mc][:], lhs,
                                     wres[pr][:, :, C:C + CD],
                                     start=False, stop=True, perf_mode=DR)
                    nc.tensor.matmul(ppi[:, j * 16:(j + 1) * 16], lhs,
                                     wres[pr][:, :, 0:C], start=False,
                                     stop=True, perf_mode=DR,
                                     skip_group_check=True)
                else:
                    nc.tensor.matmul(pmu[mc][:], lhs,
                                     wres[pr][:, :, C:C + CD],
                                     start=first, stop=False, perf_mode=DR)
                    nc.tensor.matmul(psg[mc][:], lhs,
                                     wres[pr][:, :, C + CD:],
                                     start=first, stop=False, perf_mode=DR)
                    nc.tensor.matmul(ppi[:, j * 16:(j + 1) * 16], lhs,
                                     wres[pr][:, :, 0:C],
                                     start=(first and j == 0), stop=False,
                                     perf_mode=DR, skip_group_check=True)

        for s, mcs in enumerate(SWEEPS):
            ppi = pp.tile([128, 16 * len(mcs)], F32, name=f"ppi{s}",
                          tag="ppi", bufs=2)
            pmu, psg = {}, {}
            for j, mc in enumerate(mcs):
                pmu[mc] = pp.tile([128, CD], F32, name=f"pmu{s}_{j}",
                                  tag="pmu", bufs=3)
                psg[mc] = pp.tile([128, CD], F32, name=f"psg{s}_{j}",
                                  tag="psg", bufs=3)

            # Carried pairs from the previous sweep run chunk-major, so the
            # PE restarts on chunk 0 the moment its drain adds retire.
            start_pr = 0
            if s > 0:
                for j, mc in enumerate(mcs):
                    for pr in range(NCARRY):
                        pair_matmuls(carry_z[(s, pr)], pr, mcs, pmu, psg,
                                     ppi, chunks=[(j, mc)])
                for pr in range(NCARRY):
                    del carry_z[(s, pr)]
                start_pr = NCARRY

            for pr in range(start_pr, NPAIR):
                if s == 0:
                    # Keep the resident-W queue ~8 pairs ahead of the PE.
                    if pr + 8 < NPAIR:
                        nc.sync.dma_start(wres[pr + 8][:], w_d[pr + 8])
                    if pr == 8:
                        issue_gb(0, 2)
                    elif pr == 24:
                        issue_gb(0, 3)
                else:
                    if pr == 6:
                        issue_gb(s, 1)
                    elif pr == 16:
                        issue_gb(s, 2)
                    elif pr == 30:
                        issue_gb(s, 3)
                if pr == (52 if s == 0 else 30):
                    # One bias tile per sweep; late in sweep 1 so its
                    # transfer never stalls the W-paced stream.
                    bt = bp.tile([128, len(mcs), OUT_W], F32, name=f"bt{s}",
                                 tag="bt")
                    nc.sync.dma_start(bt[:],
                                      bias_d[:, mcs[0]:mcs[0] + len(mcs), :])
                if s + 1 < len(SWEEPS):
                    if pr == 40:
                        issue_gb(s + 1, 0)
                    elif pr >= 52 and pr % 2 == 0:
                        cpr = (pr - 52) // 2
                        carry_z[(s + 1, cpr)] = gen_z(s + 1, cpr,
                                                      SWEEPS[s + 1])
                zt = gen_z(s, pr, mcs)
                pair_matmuls(zt, pr, mcs, pmu, psg, ppi)

            # Drain, phase-batched so ACT runs exp,exp,..,ln,ln.
            # softplus(v) = ln(exp(v) + 1); the reference's +1e-7 is dropped
            # (5e-7 relative effect, far below fp8 noise).
            ots, ets = {}, {}
            for j, mc in enumerate(mcs):
                # Per chunk: ei-add (feeds ACT) then mu-add; together they
                # free this chunk's psum slots for the next sweep.
                ei = op.tile([128, CD], F32, name=f"ei{s}_{j}", tag="ei",
                             bufs=3)
                nc.vector.tensor_add(ei[:], psg[mc][:], bt[:, j, C + CD:])
                ot = op.tile([128, OUT_W], F32, name=f"ot{s}_{j}", tag="ot")
                nc.vector.tensor_add(ot[:, C:C + CD], pmu[mc][:],
                                     bt[:, j, C:C + CD])
                ots[mc] = ot
                ets[mc] = ei
            for j, mc in enumerate(mcs):
                et = op.tile([128, CD], F32, name=f"et{s}_{j}", tag="et",
                             bufs=3)
                nc.scalar.activation(et[:], ets[mc][:],
                                     mybir.ActivationFunctionType.Exp)
                ets[mc] = et
            for j, mc in enumerate(mcs):
                ot = ots[mc]
                nc.vector.tensor_add(ot[:, 0:C], ppi[:, j * 16:(j + 1) * 16],
                                     bt[:, j, 0:C])
                # Output writes ride the ACT HWDGE queue: they depend on
                # compute, and on a load queue they would head-of-line
                # block the next sweep's tiles.
                nc.scalar.dma_start(out_d[:, mc, 0:C + CD], ot[:, 0:C + CD])
            for j, mc in enumerate(mcs):
                ot = ots[mc]
                nc.scalar.activation(ot[:, C + CD:], ets[mc][:],
                                     mybir.ActivationFunctionType.Ln,
                                     bias=1.0)
                nc.scalar.dma_start(out_d[:, mc, C + CD:], ot[:, C + CD:])

    nc.compile()
    _cache["nc"] = nc
    return nc


def _prep_shared(W_mu, b_mu, W_sigma, b_sigma, W_pi, b_pi):
    fp8 = ml_dtypes.float8_e4m3
    # Column order matches the reference output: [logits | loc | scale].
    w_cat = np.concatenate([W_pi, W_mu, W_sigma], axis=-1)      # [G, I, 1040]
    # K-tile pairs for DoubleRow: [pair, partition, 2, out] where
    # w_np[pr, p, i, :] = W row k = (2*pr+i)*128 + p.
    w_np = np.ascontiguousarray(
        w_cat.reshape(NPAIR, 2, 128, OUT_W).transpose(0, 2, 1, 3)
        .astype(fp8))
    b_cat = np.concatenate([b_pi, b_mu, b_sigma],
                           axis=-1).astype(np.float32)          # [G, 1040]
    return w_np, b_cat


def _core_inputs(x, g, w_np, b_cat, c):
    bf16 = ml_dtypes.bfloat16
    xs = x[c * BLOC:(c + 1) * BLOC]
    gs = g[c * BLOC:(c + 1) * BLOC]
    # x^T packed [partition, i-block, sample]: xt[p, ib, b] = x[b, ib*128+p]
    xT = np.ascontiguousarray(
        xs.T.astype(bf16).reshape(4, 128, BLOC).transpose(1, 0, 2))
    # gates broadcast across partitions: gb[p, g, b] = g[b, g]
    gT = gs.T.astype(bf16)                                      # [32, 1024]
    gb = np.ascontiguousarray(
        np.broadcast_to(gT[None, :, :], (128, G, BLOC)))        # [128,32,1024]
    # bias packed [partition, chunk, out]: bias[p, mc, o] = (g @ b)[mc*128+p, o]
    bias = np.ascontiguousarray(
        (gs.astype(np.float32) @ b_cat).reshape(NMC, 128, OUT_W)
        .transpose(1, 0, 2))
    return {"xt": xT, "gb": gb, "w": w_np, "bias": bias}


def kernel(x, g, W_mu, b_mu, W_sigma, b_sigma, W_pi, b_pi):
    nc = _build_program()
    w_np, b_cat = _prep_shared(W_mu, b_mu, W_sigma, b_sigma, W_pi, b_pi)
    in_maps = [_core_inputs(x, g, w_np, b_cat, c) for c in range(NCORES)]
    res = run_bass_kernel_spmd(nc, in_maps, core_ids=list(range(NCORES)))
    out = np.concatenate(
        [res.results[c]["out"].transpose(1, 0, 2).reshape(BLOC, OUT_W)
         for c in range(NCORES)], axis=0)
    return np.ascontiguousarray(out.astype(np.float32))


# revision 14
# speedup vs baseline: 1.0187x; 1.0187x over previous
"""GroupGMM Trainium2 kernel (fp8 DoubleRow, resident weights).

Computes, for B=8192 samples with soft group-mixture weights over G=32 groups:
    logits = einsum("bi,gio,bg->bo", x, W_pi, g) + g @ b_pi        [B, 16]
    loc    = einsum(... W_mu ...)   + g @ b_mu                     [B, 512]
    scale  = softplus(einsum(... W_sigma ...) + g @ b_sigma)+1e-7  [B, 512]
    out    = concat([logits, loc, scale], -1)                      [B, 1040]

Strategy: data-parallel over batch across 8 NeuronCores (1024 rows each).
The group einsum folds into one matmul with contraction K = G*I = 16384 via
z[b,(g,i)] = g[b,g] * x[b,i], run in fp8e4 (e4m3) with the PE's DoubleRow
perf mode (two 128-row K-tiles per pass at 0.5 cycles/row; measured overall
rel err ~1e-2 vs the 2e-2 gate). At fp8-DR speed the kernel sits on the
cost model's ridge: PE needs ~217ns per K-pair-chunk, the single 360GB/s
DMA_ENGINES resource needs ~100us for the minimal HBM traffic, and every
DMA dispatch costs ~0.7us of its sequencer (HWDGE) or ~1us of the Pool
engine (SWDGE). Layout of the work:
  - The 17MB fp8 weight tensor is RESIDENT, loaded once during sweep 1 as
    64 per-pair [128, 2, 1040] tiles on the sync HWDGE queue (a DoubleRow
    matmul's moving operand must come from a tile whose partition stride
    equals the slice free size — slicing a big 4D tile silently reads the
    wrong addresses in BIRSim — and re-streaming per sweep would triple W
    traffic). Sweep 1 is therefore W-stream-paced (~55us); sweeps 2-3 are
    PE-paced.
  - Gates load as per-sweep [128, 8-groups, mw] column-slice tiles on the
    gpsimd SWDGE queue (keeps the HWDGE/W stream clean); bias as one
    [128, chunks, 1040] tile per sweep and x^T as one resident
    [128, 4, BLOC] tile on the sync queue, sliced small at startup so the
    first matmul issues at ~3us. Output writes ride the ACT HWDGE queue so
    a compute-dependent store can never head-of-line-block a load.
  - z tiles are built per K-pair: ONE DVE multiply [128, 2, mw] bf16 (x^T
    pair slice x gate row broadcast by a stride-0 AP — 2x DVE mode), then
    cast bf16->fp8 on a rotating engine (8 ACT / 6 Pool / 2 DVE per 16)
    because a DVE multiply cannot write fp8 at 2x and no single helper
    engine can match the PE pace. GPSIMD cannot touch PSUM, so all drain
    adds stay on DVE.
Per 128-sample chunk each pair issues 3 DR matmuls (mu 512, sigma 512,
pi 16 cols; lhsT [128, 2, 128]) — the shapes concourse's tile_matmul
emits. PSUM: 3 chunks x (mu+sg) + 2 pi banks = 8, so the batch runs in 3
sweeps ([0..2],[3..5],[6..7]). Sweep boundaries pipeline: the last 12
pairs of a sweep interleave generation of the next sweep's first 6 z
tiles, and those 6 pairs are emitted chunk-major so the PE restarts on
chunk 0 as soon as its two drain adds retire. The bias term g @ b_cat is
precomputed on the host in f32 and added at drain time on DVE; sigma gets
softplus via ACT Exp+Ln.
"""

import numpy as np
import ml_dtypes

import concourse.bass as bass
import concourse.tile as tile
from concourse import bacc, mybir
from concourse.bass_utils import run_bass_kernel_spmd

B, I, G, C, D = 8192, 512, 32, 16, 32
CD = C * D                      # 512
OUT_W = C + 2 * CD              # 1040
NCORES = 8
BLOC = B // NCORES              # 1024
KTOT = G * I                    # 16384
NKT = KTOT // 128               # 128 K-tiles
NPAIR = NKT // 2                # 64 DoubleRow K-tile pairs
NMC = BLOC // 128               # 8 sample chunks per core
SWEEPS = [[0, 1, 2], [3, 4, 5], [6, 7]]
NCARRY = 6                      # pairs pre-generated across a sweep boundary

BF16 = mybir.dt.bfloat16
F32 = mybir.dt.float32
FP8 = mybir.dt.float8e4
DR = mybir.MatmulPerfMode.DoubleRow

# Rotating engine for the bf16->fp8 z cast (by pair index mod 16).
_CAST_DVE = {5, 13}
_CAST_POOL = {1, 3, 7, 9, 11, 15}

_cache: dict = {}


def _build_program():
    if "nc" in _cache:
        return _cache["nc"]
    from contextlib import ExitStack

    nc = bacc.Bacc("TRN2", target_bir_lowering=False, debug=False)

    xt_d = nc.dram_tensor("xt", [128, 4, BLOC], BF16, kind="ExternalInput")
    gb_d = nc.dram_tensor("gb", [128, G, BLOC], BF16, kind="ExternalInput")
    w_d = nc.dram_tensor("w", [NPAIR, 128, 2, OUT_W], FP8, kind="ExternalInput")
    bias_d = nc.dram_tensor("bias", [128, NMC, OUT_W], F32,
                            kind="ExternalInput")
    out_d = nc.dram_tensor("out", [128, NMC, OUT_W], F32,
                           kind="ExternalOutput")

    with tile.TileContext(nc) as tc, ExitStack() as ctx:
        res = ctx.enter_context(tc.tile_pool(name="res", bufs=1))
        gp = ctx.enter_context(tc.tile_pool(name="gp", bufs=3))
        zbp = ctx.enter_context(tc.tile_pool(name="zbp", bufs=5))
        zp = ctx.enter_context(tc.tile_pool(name="zp", bufs=8))
        op = ctx.enter_context(tc.tile_pool(name="op", bufs=3))
        bp = ctx.enter_context(tc.tile_pool(name="bp", bufs=1))
        pp = ctx.enter_context(tc.tile_pool(name="pp", bufs=1, space="PSUM"))

        # ---- startup loads ----
        # Small first slices so pair 0's z-mul starts ~1.5us in: x^T blocks
        # 0-3 for sweep-1 columns only (odd pairs use blocks 2-3!), gates
        # for groups 0-1. The x^T remainder (columns for sweeps 2-3) loads
        # mid-sweep-1 so it never delays the W stream.
        xt4 = res.tile([128, 4, BLOC], BF16, name="xt4", tag="xt4")
        nc.sync.dma_start(xt4[:, 0:2, 0:384], xt_d[:, 0:2, 0:384])
        nc.sync.dma_start(xt4[:, 2:4, 0:384], xt_d[:, 2:4, 0:384])

        gbt: dict = {}

        def issue_gb(s, t, split=False, q=None):
            # Sweep-0 gate tiles ride the gpsimd SWDGE queue (sync is busy
            # streaming W); later sweeps use the then-idle sync queue so
            # Pool's cast backlog can never delay a gate load.
            if q is None:
                q = nc.gpsimd if s == 0 else nc.sync
            mcs = SWEEPS[s]
            m0 = mcs[0] * 128
            mw = len(mcs) * 128
            tl = gp.tile([128, 8, mw], BF16, name=f"gb{s}_{t}", tag="gbs")
            if split:
                q.dma_start(tl[:, 0:2, :],
                            gb_d[:, t * 8:t * 8 + 2, m0:m0 + mw])
                q.dma_start(tl[:, 2:8, :],
                            gb_d[:, t * 8 + 2:(t + 1) * 8, m0:m0 + mw])
            else:
                q.dma_start(tl[:], gb_d[:, t * 8:(t + 1) * 8, m0:m0 + mw])
            gbt[(s, t)] = tl

        issue_gb(0, 0, split=True)

        # Resident W: 64 individual [128, 2, OUT_W] tiles on the sync queue.
        wres = [res.tile([128, 2, OUT_W], FP8, name=f"w{pr}", tag=f"w{pr}")
                for pr in range(NPAIR)]
        for pr in range(8):
            nc.sync.dma_start(wres[pr][:], w_d[pr])
        issue_gb(0, 1)

        carry_z: dict = {}

        def gen_z(s, pr, mcs):
            gi = pr // 2
            xb0 = (pr % 2) * 2
            m0 = mcs[0] * 128
            mw = len(mcs) * 128
            gsl = gbt[(s, gi // 8)][:, gi % 8, :].unsqueeze(1).broadcast_to(
                [128, 2, mw])
            xsl = xt4[:, xb0:xb0 + 2, m0:m0 + mw]
            zt = zp.tile([128, 2, mw], FP8, name=f"zt{s}_{pr}", tag="zt")
            m = pr % 16
            if m in _CAST_DVE or (s == 0 and pr < 4):
                # Direct fp8-out multiply (1x DVE): used for the DVE share
                # and at startup, where an ACT cast would pull the first
                # matmul behind a 1.3us act-table load.
                nc.vector.tensor_mul(zt[:], xsl, gsl)
                return zt
            zb = zbp.tile([128, 2, mw], BF16, name=f"zb{s}_{pr}", tag="zb")
            nc.vector.tensor_mul(zb[:], xsl, gsl)
            if m in _CAST_POOL:
                nc.gpsimd.tensor_copy(zt[:], zb[:])
            else:
                nc.scalar.activation(zt[:], zb[:],
                                     mybir.ActivationFunctionType.Copy)
            return zt

        def pair_matmuls(zt, pr, mcs, pmu, psg, ppi, chunks=None):
            first = pr == 0
            last = pr == NPAIR - 1
            for j, mc in (chunks if chunks is not None else enumerate(mcs)):
                lhs = zt[:, :, j * 128:(j + 1) * 128]
                if last:
                    # sigma first so its drain chain starts earliest
                    nc.tensor.matmul(psg[mc][:], lhs,
                                     wres[pr][:, :, C + CD:],
                                     start=False, stop=True, perf_mode=DR)
                    nc.tensor.matmul(pmu[mc][:], lhs,
                                     wres[pr][:, :, C:C + CD],
                                     start=False, stop=True, perf_mode=DR)
                    nc.tensor.matmul(ppi[:, j * 16:(j + 1) * 16], lhs,
                                     wres[pr][:, :, 0:C], start=False,
                                     stop=True, perf_mode=DR,
                                     skip_group_check=True)
                else:
                    nc.tensor.matmul(pmu[mc][:], lhs,
                                     wres[pr][:, :, C:C + CD],
                                     start=first, stop=False, perf_mode=DR)
                    nc.tensor.matmul(psg[mc][:], lhs,
                                     wres[pr][:, :, C + CD:],
                                     start=first, stop=False, perf_mode=DR)
                    nc.tensor.matmul(ppi[:, j * 16:(j + 1) * 16], lhs,
                                     wres[pr][:, :, 0:C],
                                     start=(first and j == 0), stop=False,
                                     perf_mode=DR, skip_group_check=True)

        for s, mcs in enumerate(SWEEPS):
            ppi = pp.tile([128, 16 * len(mcs)], F32, name=f"ppi{s}",
                          tag="ppi", bufs=2)
            pmu, psg = {}, {}
            for j, mc in enumerate(mcs):
                pmu[mc] = pp.tile([128, CD], F32, name=f"pmu{s}_{j}",
                                  tag="pmu", bufs=3)
                psg[mc] = pp.tile([128, CD], F32, name=f"psg{s}_{j}",
                                  tag="psg", bufs=3)

            # Carried pairs from the previous sweep run chunk-major, so the
            # PE restarts on chunk 0 the moment its drain adds retire.
            start_pr = 0
            if s > 0:
                for j, mc in enumerate(mcs):
                    for pr in range(NCARRY):
                        pair_matmuls(carry_z[(s, pr)], pr, mcs, pmu, psg,
                                     ppi, chunks=[(j, mc)])
                for pr in range(NCARRY):
                    del carry_z[(s, pr)]
                start_pr = NCARRY

            for pr in range(start_pr, NPAIR):
                if s == 0:
                    # Keep the resident-W queue ~8 pairs ahead of the PE.
                    if pr + 8 < NPAIR:
                        nc.sync.dma_start(wres[pr + 8][:], w_d[pr + 8])
                    if pr == 8:
                        issue_gb(0, 2)
                    elif pr == 24:
                        issue_gb(0, 3)
                    elif pr == 30:
                        # x^T columns for sweeps 2-3, now that the W queue
                        # has drained far ahead of the gate/bias deadline.
                        nc.sync.dma_start(xt4[:, 0:2, 384:],
                                          xt_d[:, 0:2, 384:])
                    elif pr == 34:
                        nc.sync.dma_start(xt4[:, 2:4, 384:],
                                          xt_d[:, 2:4, 384:])
                else:
                    if pr == 6:
                        issue_gb(s, 1)
                    elif pr == 14:
                        issue_gb(s, 2)
                    elif pr == 28:
                        issue_gb(s, 3)
                if pr == (56 if s == 0 else 30):
                    # One bias tile per sweep; after the last W issue in
                    # sweep 1 so its transfer never stalls the W stream.
                    bt = bp.tile([128, len(mcs), OUT_W], F32, name=f"bt{s}",
                                 tag="bt")
                    nc.sync.dma_start(bt[:],
                                      bias_d[:, mcs[0]:mcs[0] + len(mcs), :])
                if s + 1 < len(SWEEPS):
                    if pr == 40:
                        # During sweep 0 the sync queue is still streaming W.
                        issue_gb(s + 1, 0,
                                 q=nc.gpsimd if s == 0 else nc.sync)
                    elif pr >= 52 and pr % 2 == 0:
                        cpr = (pr - 52) // 2
                        carry_z[(s + 1, cpr)] = gen_z(s + 1, cpr,
                                                      SWEEPS[s + 1])
                zt = gen_z(s, pr, mcs)
                pair_matmuls(zt, pr, mcs, pmu, psg, ppi)

            # Drain, phase-batched so ACT runs exp,exp,..,ln,ln.
            # softplus(v) = ln(exp(v) + 1); the reference's +1e-7 is dropped
            # (5e-7 relative effect, far below fp8 noise).
            ots, ets = {}, {}
            for j, mc in enumerate(mcs):
                # Per chunk: ei-add (feeds ACT) then mu-add; together they
                # free this chunk's psum slots for the next sweep.
                ei = op.tile([128, CD], F32, name=f"ei{s}_{j}", tag="ei",
                             bufs=3)
                nc.vector.tensor_add(ei[:], psg[mc][:], bt[:, j, C + CD:])
                ot = op.tile([128, OUT_W], F32, name=f"ot{s}_{j}", tag="ot")
                nc.vector.tensor_add(ot[:, C:C + CD], pmu[mc][:],
                                     bt[:, j, C:C + CD])
                ots[mc] = ot
                ets[mc] = ei
            for j, mc in enumerate(mcs):
                et = op.tile([128, CD], F32, name=f"et{s}_{j}", tag="et",
                             bufs=3)
                nc.scalar.activation(et[:], ets[mc][:],
                                     mybir.ActivationFunctionType.Exp)
                ets[mc] = et
            for j, mc in enumerate(mcs):
                ot = ots[mc]
                nc.vector.tensor_add(ot[:, 0:C], ppi[:, j * 16:(j + 1) * 16],
                                     bt[:, j, 0:C])
                # Output writes ride the ACT HWDGE queue: they depend on
                # compute, and on a load queue they would head-of-line
                # block the next sweep's tiles.
                nc.scalar.dma_start(out_d[:, mc, 0:C + CD], ot[:, 0:C + CD])
            for j, mc in enumerate(mcs):
                ot = ots[mc]
                nc.scalar.activation(ot[:, C + CD:], ets[mc][:],
                                     mybir.ActivationFunctionType.Ln,
                                     bias=1.0)
                nc.scalar.dma_start(out_d[:, mc, C + CD:], ot[:, C + CD:])

    nc.compile()
    _cache["nc"] = nc
    return nc


def _prep_shared(W_mu, b_mu, W_sigma, b_sigma, W_pi, b_pi):
    fp8 = ml_dtypes.float8_e4m3
    # Column order matches the reference output: [logits | loc | scale].
    w_cat = np.concatenate([W_pi, W_mu, W_sigma], axis=-1)      # [G, I, 1040]
    # K-tile pairs for DoubleRow: [pair, partition, 2, out] where
    # w_np[pr, p, i, :] = W row k = (2*pr+i)*128 + p.
    w_np = np.ascontiguousarray(
        w_cat.reshape(NPAIR, 2, 128, OUT_W).transpose(0, 2, 1, 3)
        .astype(fp8))
    b_cat = np.concatenate([b_pi, b_mu, b_sigma],
                           axis=-1).astype(np.float32)          # [G, 1040]
    return w_np, b_cat


def _core_inputs(x, g, w_np, b_cat, c):
    bf16 = ml_dtypes.bfloat16
    xs = x[c * BLOC:(c + 1) * BLOC]
    gs = g[c * BLOC:(c + 1) * BLOC]
    # x^T packed [partition, i-block, sample]: xt[p, ib, b] = x[b, ib*128+p]
    xT = np.ascontiguousarray(
        xs.T.astype(bf16).reshape(4, 128, BLOC).transpose(1, 0, 2))
    # gates broadcast across partitions: gb[p, g, b] = g[b, g]
    gT = gs.T.astype(bf16)                                      # [32, 1024]
    gb = np.ascontiguousarray(
        np.broadcast_to(gT[None, :, :], (128, G, BLOC)))        # [128,32,1024]
    # bias packed [partition, chunk, out]: bias[p, mc, o] = (g @ b)[mc*128+p, o]
    bias = np.ascontiguousarray(
        (gs.astype(np.float32) @ b_cat).reshape(NMC, 128, OUT_W)
        .transpose(1, 0, 2))
    return {"xt": xT, "gb": gb, "w": w_np, "bias": bias}


def kernel(x, g, W_mu, b_mu, W_sigma, b_sigma, W_pi, b_pi):
    nc = _build_program()
    w_np, b_cat = _prep_shared(W_mu, b_mu, W_sigma, b_sigma, W_pi, b_pi)
    in_maps = [_core_inputs(x, g, w_np, b_cat, c) for c in range(NCORES)]
    res = run_bass_kernel_spmd(nc, in_maps, core_ids=list(range(NCORES)))
    out = np.concatenate(
        [res.results[c]["out"].transpose(1, 0, 2).reshape(BLOC, OUT_W)
         for c in range(NCORES)], axis=0)
    return np.ascontiguousarray(out.astype(np.float32))


# revision 22
# speedup vs baseline: 1.0846x; 1.0647x over previous
"""GroupGMM Trainium2 kernel (fp8 DoubleRow, resident weights).

Computes, for B=8192 samples with soft group-mixture weights over G=32 groups:
    logits = einsum("bi,gio,bg->bo", x, W_pi, g) + g @ b_pi        [B, 16]
    loc    = einsum(... W_mu ...)   + g @ b_mu                     [B, 512]
    scale  = softplus(einsum(... W_sigma ...) + g @ b_sigma)+1e-7  [B, 512]
    out    = concat([logits, loc, scale], -1)                      [B, 1040]

Strategy: data-parallel over batch across 8 NeuronCores (1024 rows each).
The group einsum folds into one matmul with contraction K = G*I = 16384 via
z[b,(g,i)] = g[b,g] * x[b,i], run in fp8e4 (e4m3) with the PE's DoubleRow
perf mode (two 128-row K-tiles per pass at 0.5 cycles/row; measured overall
rel err ~1e-2 vs the 2e-2 gate). At fp8-DR speed the kernel sits on the
cost model's ridge: PE needs ~217ns per K-pair-chunk, the single 360GB/s
DMA_ENGINES resource needs ~100us for the minimal HBM traffic, and every
DMA dispatch costs ~0.7us of its sequencer (HWDGE) or ~1us of the Pool
engine (SWDGE). Layout of the work:
  - The 17MB fp8 weight tensor is RESIDENT, loaded once during sweep 1 as
    64 per-pair [128, 2, 1040] tiles on the sync HWDGE queue (a DoubleRow
    matmul's moving operand must come from a tile whose partition stride
    equals the slice free size — slicing a big 4D tile silently reads the
    wrong addresses in BIRSim — and re-streaming per sweep would triple W
    traffic). Sweep 1 is therefore W-stream-paced (~55us); sweeps 2-3 are
    PE-paced.
  - Gates load as per-sweep [128, 8-groups, mw] column-slice tiles on the
    gpsimd SWDGE queue (keeps the HWDGE/W stream clean); bias as one
    [128, chunks, 1040] tile per sweep and x^T as one resident
    [128, 4, BLOC] tile on the sync queue, sliced small at startup so the
    first matmul issues at ~3us. Output writes ride the ACT HWDGE queue so
    a compute-dependent store can never head-of-line-block a load.
  - z tiles are built per K-pair: ONE DVE multiply [128, 2, mw] bf16 (x^T
    pair slice x gate row broadcast by a stride-0 AP — 2x DVE mode), then
    cast bf16->fp8 on a rotating engine (8 ACT / 6 Pool / 2 DVE per 16)
    because a DVE multiply cannot write fp8 at 2x and no single helper
    engine can match the PE pace. GPSIMD cannot touch PSUM, so all drain
    adds stay on DVE.
Per 128-sample chunk each pair issues 3 DR matmuls (mu 512, sigma 512,
pi 16 cols; lhsT [128, 2, 128]) — the shapes concourse's tile_matmul
emits. PSUM: 3 chunks x (mu+sg) + 2 pi banks = 8, so the batch runs in 3
sweeps ([0..2],[3..5],[6..7]). Sweep boundaries pipeline: the last 12
pairs of a sweep interleave generation of the next sweep's first 6 z
tiles, and those 6 pairs are emitted chunk-major so the PE restarts on
chunk 0 as soon as its two drain adds retire. The bias term g @ b_cat is
precomputed on the host in f32 and added at drain time on DVE; sigma gets
softplus via ACT Exp+Ln.
"""

import numpy as np
import ml_dtypes

import concourse.bass as bass
import concourse.tile as tile
from concourse import bacc, mybir
from concourse.bass_utils import run_bass_kernel_spmd

B, I, G, C, D = 8192, 512, 32, 16, 32
CD = C * D                      # 512
OUT_W = C + 2 * CD              # 1040
NCORES = 8
BLOC = B // NCORES              # 1024
KTOT = G * I                    # 16384
NKT = KTOT // 128               # 128 K-tiles
NPAIR = NKT // 2                # 64 DoubleRow K-tile pairs
NMC = BLOC // 128               # 8 sample chunks per core
SWEEPS = [[0, 1, 2], [3, 4, 5], [6, 7]]
# Pairs pre-generated across each sweep boundary. The final 2-chunk sweep
# does less PE work per pair, so it needs a longer runway to cover the
# serial DVE drain adds of the previous sweep.
NCARRY = {1: 8, 2: 9}

BF16 = mybir.dt.bfloat16
F32 = mybir.dt.float32
FP8 = mybir.dt.float8e4
DR = mybir.MatmulPerfMode.DoubleRow

# Rotating engine for the bf16->fp8 z cast (by pair index mod 16).
# Alternating ACT/other so the slower ACT cast (825ns vs 652ns PE pace)
# never runs twice back-to-back and the z supply can't fall behind.
_CAST_DVE = {7, 15}
_CAST_POOL = {1, 3, 5, 9, 11, 13}

_cache: dict = {}


def _build_program():
    if "nc" in _cache:
        return _cache["nc"]
    from contextlib import ExitStack

    nc = bacc.Bacc("TRN2", target_bir_lowering=False, debug=False)

    xt_d = nc.dram_tensor("xt", [128, 4, BLOC], BF16, kind="ExternalInput")
    gb_d = nc.dram_tensor("gb", [128, G, BLOC], BF16, kind="ExternalInput")
    w_d = nc.dram_tensor("w", [NPAIR, 128, 2, OUT_W], FP8, kind="ExternalInput")
    bias_d = nc.dram_tensor("bias", [128, NMC, OUT_W], F32,
                            kind="ExternalInput")
    out_d = nc.dram_tensor("out", [128, NMC, OUT_W], F32,
                           kind="ExternalOutput")

    with tile.TileContext(nc) as tc, ExitStack() as ctx:
        res = ctx.enter_context(tc.tile_pool(name="res", bufs=1))
        gp = ctx.enter_context(tc.tile_pool(name="gp", bufs=3))
        zbp = ctx.enter_context(tc.tile_pool(name="zbp", bufs=4))
        zp = ctx.enter_context(tc.tile_pool(name="zp", bufs=10))
        op = ctx.enter_context(tc.tile_pool(name="op", bufs=3))
        bp = ctx.enter_context(tc.tile_pool(name="bp", bufs=1))
        pp = ctx.enter_context(tc.tile_pool(name="pp", bufs=1, space="PSUM"))

        # ---- startup loads ----
        # Small first slices so pair 0's z-mul starts ~1.5us in: x^T blocks
        # 0-3 for sweep-1 columns only (odd pairs use blocks 2-3!), gates
        # for groups 0-1. The x^T remainder (columns for sweeps 2-3) loads
        # mid-sweep-1 so it never delays the W stream.
        xt4 = res.tile([128, 4, BLOC], BF16, name="xt4", tag="xt4")
        nc.sync.dma_start(xt4[:, 0:2, 0:384], xt_d[:, 0:2, 0:384])
        nc.sync.dma_start(xt4[:, 2:4, 0:384], xt_d[:, 2:4, 0:384])

        gbt: dict = {}

        def issue_gb(s, t, split=False, q=None):
            # Sweep-0 gate tiles ride the gpsimd SWDGE queue (sync is busy
            # streaming W); later sweeps use the then-idle sync queue so
            # Pool's cast backlog can never delay a gate load.
            if q is None:
                q = nc.gpsimd if s == 0 else nc.sync
            mcs = SWEEPS[s]
            m0 = mcs[0] * 128
            mw = len(mcs) * 128
            tl = gp.tile([128, 8, mw], BF16, name=f"gb{s}_{t}", tag="gbs")
            if split:
                q.dma_start(tl[:, 0:2, :],
                            gb_d[:, t * 8:t * 8 + 2, m0:m0 + mw])
                q.dma_start(tl[:, 2:8, :],
                            gb_d[:, t * 8 + 2:(t + 1) * 8, m0:m0 + mw])
            else:
                q.dma_start(tl[:], gb_d[:, t * 8:(t + 1) * 8, m0:m0 + mw])
            gbt[(s, t)] = tl

        issue_gb(0, 0, split=True)

        # Resident W: 64 individual [128, 2, OUT_W] tiles on the sync queue.
        wres = [res.tile([128, 2, OUT_W], FP8, name=f"w{pr}", tag=f"w{pr}")
                for pr in range(NPAIR)]
        for pr in range(8):
            nc.sync.dma_start(wres[pr][:], w_d[pr])
        issue_gb(0, 1)

        carry_z: dict = {}

        def gen_z(s, pr, mcs):
            gi = pr // 2
            xb0 = (pr % 2) * 2
            m0 = mcs[0] * 128
            mw = len(mcs) * 128
            gsl = gbt[(s, gi // 8)][:, gi % 8, :].unsqueeze(1).broadcast_to(
                [128, 2, mw])
            xsl = xt4[:, xb0:xb0 + 2, m0:m0 + mw]
            zt = zp.tile([128, 2, mw], FP8, name=f"zt{s}_{pr}", tag="zt")
            m = pr % 16
            if m in _CAST_DVE or (s == 0 and pr < 4):
                # Direct fp8-out multiply (1x DVE): used for the DVE share
                # and at startup, where an ACT cast would pull the first
                # matmul behind a 1.3us act-table load.
                nc.vector.tensor_mul(zt[:], xsl, gsl)
                return zt
            zb = zbp.tile([128, 2, mw], BF16, name=f"zb{s}_{pr}", tag="zb")
            nc.vector.tensor_mul(zb[:], xsl, gsl)
            if m in _CAST_POOL:
                nc.gpsimd.tensor_copy(zt[:], zb[:])
            else:
                nc.scalar.activation(zt[:], zb[:],
                                     mybir.ActivationFunctionType.Copy)
            return zt

        def pair_matmuls(zt, pr, mcs, pmu, psg, ppi, chunks=None):
            first = pr == 0
            last = pr == NPAIR - 1
            for j, mc in (chunks if chunks is not None else enumerate(mcs)):
                lhs = zt[:, :, j * 128:(j + 1) * 128]
                if last:
                    # sigma first so its drain chain starts earliest
                    nc.tensor.matmul(psg[mc][:], lhs,
                                     wres[pr][:, :, C + CD:],
                                     start=False, stop=True, perf_mode=DR)
                    nc.tensor.matmul(pmu[mc][:], lhs,
                                     wres[pr][:, :, C:C + CD],
                                     start=False, stop=True, perf_mode=DR)
                    nc.tensor.matmul(ppi[:, j * 16:(j + 1) * 16], lhs,
                                     wres[pr][:, :, 0:C], start=False,
                                     stop=True, perf_mode=DR,
                                     skip_group_check=True)
                else:
                    nc.tensor.matmul(pmu[mc][:], lhs,
                                     wres[pr][:, :, C:C + CD],
                                     start=first, stop=False, perf_mode=DR)
                    nc.tensor.matmul(psg[mc][:], lhs,
                                     wres[pr][:, :, C + CD:],
                                     start=first, stop=False, perf_mode=DR)
                    nc.tensor.matmul(ppi[:, j * 16:(j + 1) * 16], lhs,
                                     wres[pr][:, :, 0:C],
                                     start=(first and j == 0), stop=False,
                                     perf_mode=DR, skip_group_check=True)

        for s, mcs in enumerate(SWEEPS):
            ppi = pp.tile([128, 16 * len(mcs)], F32, name=f"ppi{s}",
                          tag="ppi", bufs=2)
            pmu, psg = {}, {}
            for j, mc in enumerate(mcs):
                pmu[mc] = pp.tile([128, CD], F32, name=f"pmu{s}_{j}",
                                  tag="pmu", bufs=3)
                psg[mc] = pp.tile([128, CD], F32, name=f"psg{s}_{j}",
                                  tag="psg", bufs=3)

            # Carried pairs from the previous sweep run chunk-major, so the
            # PE restarts on chunk 0 the moment its drain adds retire.
            start_pr = 0
            if s > 0:
                nc_s = NCARRY[s]
                for j, mc in enumerate(mcs):
                    for pr in range(nc_s):
                        pair_matmuls(carry_z[(s, pr)], pr, mcs, pmu, psg,
                                     ppi, chunks=[(j, mc)])
                for pr in range(nc_s):
                    del carry_z[(s, pr)]
                start_pr = nc_s

            for pr in range(start_pr, NPAIR):
                if s == 0:
                    # Keep the resident-W queue ~8 pairs ahead of the PE.
                    if pr + 8 < NPAIR:
                        nc.sync.dma_start(wres[pr + 8][:], w_d[pr + 8])
                    if pr == 8:
                        issue_gb(0, 2)
                    elif pr == 24:
                        issue_gb(0, 3)
                    elif pr == 30:
                        # x^T columns for sweeps 2-3, now that the W queue
                        # has drained far ahead of the gate/bias deadline.
                        nc.sync.dma_start(xt4[:, 0:2, 384:],
                                          xt_d[:, 0:2, 384:])
                    elif pr == 34:
                        nc.sync.dma_start(xt4[:, 2:4, 384:],
                                          xt_d[:, 2:4, 384:])
                else:
                    if pr == 8:
                        issue_gb(s, 2)
                    elif pr == 20:
                        issue_gb(s, 3)
                if pr == (56 if s == 0 else 30):
                    # One bias tile per sweep; after the last W issue in
                    # sweep 1 so its transfer never stalls the W stream.
                    bt = bp.tile([128, len(mcs), OUT_W], F32, name=f"bt{s}",
                                 tag="bt")
                    nc.sync.dma_start(bt[:],
                                      bias_d[:, mcs[0]:mcs[0] + len(mcs), :])
                if s + 1 < len(SWEEPS):
                    q_pref = nc.gpsimd if s == 0 else nc.sync
                    if pr == 40:
                        # During sweep 0 the sync queue is still streaming W.
                        issue_gb(s + 1, 0, q=q_pref)
                    elif pr == 48:
                        # Next sweep's second gate tile too: issued mid-next-
                        # sweep it would race the pair-16 deadline.
                        issue_gb(s + 1, 1, q=q_pref)
                    c0 = NPAIR - 2 * NCARRY[s + 1]
                    if pr >= c0 and (pr - c0) % 2 == 0:
                        cpr = (pr - c0) // 2
                        carry_z[(s + 1, cpr)] = gen_z(s + 1, cpr,
                                                      SWEEPS[s + 1])
                zt = gen_z(s, pr, mcs)
                pair_matmuls(zt, pr, mcs, pmu, psg, ppi)

            # Drain. softplus(v) = ln(exp(v) + 1); the reference's +1e-7 is
            # dropped (5e-7 relative effect, far below fp8 noise). The Exp
            # and Ln over all chunks are each ONE wide ACT op: the scheduler
            # cannot interleave them (each interleave costs a 1.3us act-
            # table reload), and the tail shrinks to add->Exp->Ln->store.
            nmc_s = len(mcs)
            ots = {}
            eiT = op.tile([128, nmc_s * CD], F32, name=f"ei{s}", tag="ei",
                          bufs=1)
            for j, mc in enumerate(mcs):
                # Per chunk: ei-add (feeds ACT) then mu-add; together they
                # free this chunk's psum slots for the next sweep.
                nc.vector.tensor_add(eiT[:, j * CD:(j + 1) * CD], psg[mc][:],
                                     bt[:, j, C + CD:])
                ot = op.tile([128, C + CD], F32, name=f"ot{s}_{j}", tag="ot")
                nc.vector.tensor_add(ot[:, C:C + CD], pmu[mc][:],
                                     bt[:, j, C:C + CD])
                ots[mc] = ot
            etT = op.tile([128, nmc_s * CD], F32, name=f"et{s}", tag="et",
                          bufs=1)
            nc.scalar.activation(etT[:], eiT[:],
                                 mybir.ActivationFunctionType.Exp)
            for j, mc in enumerate(mcs):
                ot = ots[mc]
                nc.vector.tensor_add(ot[:, 0:C], ppi[:, j * 16:(j + 1) * 16],
                                     bt[:, j, 0:C])
                # Dispatch on the gpsimd queue: on the ACT queue this store
                # would park at the queue head waiting for the DVE pi-add
                # and block the Ln behind it.
                nc.gpsimd.dma_start(out_d[:, mc, 0:C + CD], ot[:])
            lnT = op.tile([128, nmc_s * CD], F32, name=f"ln{s}", tag="ln",
                          bufs=1)
            nc.scalar.activation(lnT[:], etT[:],
                                 mybir.ActivationFunctionType.Ln, bias=1.0)
            for j, mc in enumerate(mcs):
                nc.scalar.dma_start(out_d[:, mc, C + CD:],
                                    lnT[:, j * CD:(j + 1) * CD])

    nc.compile()
    _cache["nc"] = nc
    return nc


def _prep_shared(W_mu, b_mu, W_sigma, b_sigma, W_pi, b_pi):
    fp8 = ml_dtypes.float8_e4m3
    # Column order matches the reference output: [logits | loc | scale].
    w_cat = np.concatenate([W_pi, W_mu, W_sigma], axis=-1)      # [G, I, 1040]
    # K-tile pairs for DoubleRow: [pair, partition, 2, out] where
    # w_np[pr, p, i, :] = W row k = (2*pr+i)*128 + p.
    w_np = np.ascontiguousarray(
        w_cat.reshape(NPAIR, 2, 128, OUT_W).transpose(0, 2, 1, 3)
        .astype(fp8))
    b_cat = np.concatenate([b_pi, b_mu, b_sigma],
                           axis=-1).astype(np.float32)          # [G, 1040]
    return w_np, b_cat


def _core_inputs(x, g, w_np, b_cat, c):
    bf16 = ml_dtypes.bfloat16
    xs = x[c * BLOC:(c + 1) * BLOC]
    gs = g[c * BLOC:(c + 1) * BLOC]
    # x^T packed [partition, i-block, sample]: xt[p, ib, b] = x[b, ib*128+p]
    xT = np.ascontiguousarray(
        xs.T.astype(bf16).reshape(4, 128, BLOC).transpose(1, 0, 2))
    # gates broadcast across partitions: gb[p, g, b] = g[b, g]
    gT = gs.T.astype(bf16)                                      # [32, 1024]
    gb = np.ascontiguousarray(
        np.broadcast_to(gT[None, :, :], (128, G, BLOC)))        # [128,32,1024]
    # bias packed [partition, chunk, out]: bias[p, mc, o] = (g @ b)[mc*128+p, o]
    bias = np.ascontiguousarray(
        (gs.astype(np.float32) @ b_cat).reshape(NMC, 128, OUT_W)
        .transpose(1, 0, 2))
    return {"xt": xT, "gb": gb, "w": w_np, "bias": bias}


def kernel(x, g, W_mu, b_mu, W_sigma, b_sigma, W_pi, b_pi):
    nc = _build_program()
    w_np, b_cat = _prep_shared(W_mu, b_mu, W_sigma, b_sigma, W_pi, b_pi)
    in_maps = [_core_inputs(x, g, w_np, b_cat, c) for c in range(NCORES)]
    res = run_bass_kernel_spmd(nc, in_maps, core_ids=list(range(NCORES)))
    out = np.concatenate(
        [res.results[c]["out"].transpose(1, 0, 2).reshape(BLOC, OUT_W)
         for c in range(NCORES)], axis=0)
    return np.ascontiguousarray(out.astype(np.float32))


# revision 43
# speedup vs baseline: 1.1042x; 1.0180x over previous
"""GroupGMM Trainium2 kernel (fp8 DoubleRow, resident weights).

Computes, for B=8192 samples with soft group-mixture weights over G=32 groups:
    logits = einsum("bi,gio,bg->bo", x, W_pi, g) + g @ b_pi        [B, 16]
    loc    = einsum(... W_mu ...)   + g @ b_mu                     [B, 512]
    scale  = softplus(einsum(... W_sigma ...) + g @ b_sigma)+1e-7  [B, 512]
    out    = concat([logits, loc, scale], -1)                      [B, 1040]

Strategy: data-parallel over batch across 8 NeuronCores (1024 rows each).
The group einsum folds into one matmul with contraction K = G*I = 16384 via
z[b,(g,i)] = g[b,g] * x[b,i], run in fp8e4 (e4m3) with the PE's DoubleRow
perf mode (two 128-row K-tiles per pass at 0.5 cycles/row; measured overall
rel err ~1e-2 vs the 2e-2 gate). At fp8-DR speed the kernel sits on the
cost model's ridge: the PE needs ~217ns per K-pair-chunk, the single
360GB/s DMA_ENGINES resource needs ~100us for the minimal HBM traffic, and
every DMA dispatch costs ~0.7us of sequencer (HWDGE) or ~1us of Pool
engine (SWDGE). Layout of the work:
  - The 17MB fp8 weight tensor is RESIDENT, loaded once during sweep 1 as
    64 per-pair [128, 2, 1040] tiles on the sync HWDGE queue (a DoubleRow
    matmul's MOVING operand must come from a tile whose partition stride
    equals the slice free size — slicing a big 4D tile silently reads the
    wrong addresses in BIRSim — and re-streaming W per sweep would triple
    its traffic). Sweep 1 is therefore W-stream-paced; sweeps 2-3 are
    PE-paced.
  - z tiles are built per K-pair: ONE DVE multiply [128, 2, mw] bf16 (x^T
    pair slice x gate row broadcast by a stride-0 AP — 2x DVE mode), then
    cast bf16->fp8 on a rotating engine (8 ACT / 6 Pool / 2 DVE per 16,
    strictly alternating ACT with the others) because a DVE multiply
    cannot write fp8 at 2x and no single helper engine can match the PE
    pace. Every PE stall also costs a ~3us pstate ramp at half clock, so
    the z supply keeps a deep (14-tile) run-ahead. GPSIMD cannot touch
    PSUM, so all drain adds stay on DVE.
  - Gates load as per-sweep [128, 8-groups, mw] column slices (sweep-0 on
    the gpsimd SWDGE queue, later sweeps on the then-idle sync queue);
    bias as one [128, chunks, 1040] tile per sweep, issued late in sweep 1
    so it never stalls the W stream; x^T as one resident [128, 4, BLOC]
    tile loaded in column pieces as sweeps need them. Mu/pi stores ride
    the sync queue, scale stores the ACT queue — each dispatched only when
    its data is ready so no store can head-of-line-block a load or an Ln.
Per 128-sample chunk each pair issues 3 DR matmuls (mu 512, sigma 512,
pi 16 cols; lhsT [128, 2, 128]). PSUM: 3 chunks x (mu+sg) + 2 pi banks =
8 banks, so the batch runs in 3 sweeps ([0..2],[3..5],[6..7]). Sweep
boundaries pipeline: the tail of each sweep pre-generates the next sweep's
first pairs, which are then emitted chunk-major so the PE restarts on
chunk 0 the moment its two drain adds retire. The bias term g @ b_cat is
precomputed on the host in f32 and added at drain time on DVE; sigma gets
softplus via one wide ACT Exp and one wide Ln per sweep (split per chunk
in the final sweep to overlap the last stores).
"""

import numpy as np
import ml_dtypes

import concourse.bass as bass
import concourse.tile as tile
from concourse import bacc, mybir
from concourse.bass_utils import run_bass_kernel_spmd

B, I, G, C, D = 8192, 512, 32, 16, 32
CD = C * D                      # 512
OUT_W = C + 2 * CD              # 1040
NCORES = 8
BLOC = B // NCORES              # 1024
KTOT = G * I                    # 16384
NKT = KTOT // 128               # 128 K-tiles
NPAIR = NKT // 2                # 64 DoubleRow K-tile pairs
NMC = BLOC // 128               # 8 sample chunks per core
SWEEPS = [[0, 1, 2], [3, 4, 5], [6, 7]]
# Pairs pre-generated across each sweep boundary. The final 2-chunk sweep
# does less PE work per pair, so it needs a longer runway to cover the
# serial DVE drain adds of the previous sweep.
NCARRY = {1: 8, 2: 12}

BF16 = mybir.dt.bfloat16
F32 = mybir.dt.float32
FP8 = mybir.dt.float8e4
DR = mybir.MatmulPerfMode.DoubleRow

# Rotating engine for the bf16->fp8 z cast (by pair index mod 16):
# alternating ACT/other so the slower ACT cast never runs twice
# back-to-back and the z supply can't fall behind.
_CAST_DVE = {7, 15}
_CAST_POOL = {1, 3, 5, 9, 11, 13}

_cache: dict = {}


def _build_program():
    if "nc" in _cache:
        return _cache["nc"]
    from contextlib import ExitStack

    nc = bacc.Bacc("TRN2", target_bir_lowering=False, debug=False)

    xt_d = nc.dram_tensor("xt", [128, 4, BLOC], BF16, kind="ExternalInput")
    gb_d = nc.dram_tensor("gb", [128, G, BLOC], BF16, kind="ExternalInput")
    w_d = nc.dram_tensor("w", [NPAIR, 128, 2, OUT_W], FP8, kind="ExternalInput")
    bias_d = nc.dram_tensor("bias", [128, NMC, OUT_W], F32,
                            kind="ExternalInput")
    out_d = nc.dram_tensor("out", [128, NMC, OUT_W], F32,
                           kind="ExternalOutput")

    with tile.TileContext(nc) as tc, ExitStack() as ctx:
        res = ctx.enter_context(tc.tile_pool(name="res", bufs=1))
        gp = ctx.enter_context(tc.tile_pool(name="gp", bufs=3))
        zbp = ctx.enter_context(tc.tile_pool(name="zbp", bufs=3))
        zp = ctx.enter_context(tc.tile_pool(name="zp", bufs=14))
        op = ctx.enter_context(tc.tile_pool(name="op", bufs=3))
        bp = ctx.enter_context(tc.tile_pool(name="bp", bufs=1))
        pp = ctx.enter_context(tc.tile_pool(name="pp", bufs=1, space="PSUM"))

        # ---- startup loads ----
        # Small first slices so pair 0's z-mul starts ~1.5us in: x^T blocks
        # 0-3 for sweep-1 columns only (odd pairs use blocks 2-3!), gates
        # for groups 0-1. The x^T remainder (columns for sweeps 2-3) loads
        # mid-sweep-1 so it never delays the W stream.
        xt4 = res.tile([128, 4, BLOC], BF16, name="xt4", tag="xt4")
        nc.sync.dma_start(xt4[:, 0:2, 0:384], xt_d[:, 0:2, 0:384])
        nc.sync.dma_start(xt4[:, 2:4, 0:384], xt_d[:, 2:4, 0:384])

        gbt: dict = {}

        def issue_gb(s, t, split=False, q=None):
            # Sweep-0 gate tiles ride the gpsimd SWDGE queue (sync is busy
            # streaming W); later sweeps use the then-idle sync queue so
            # Pool's cast backlog can never delay a gate load.
            if q is None:
                q = nc.gpsimd if s == 0 else nc.sync
            mcs = SWEEPS[s]
            m0 = mcs[0] * 128
            mw = len(mcs) * 128
            tl = gp.tile([128, 8, mw], BF16, name=f"gb{s}_{t}", tag="gbs")
            if split:
                q.dma_start(tl[:, 0:2, :],
                            gb_d[:, t * 8:t * 8 + 2, m0:m0 + mw])
                q.dma_start(tl[:, 2:8, :],
                            gb_d[:, t * 8 + 2:(t + 1) * 8, m0:m0 + mw])
            else:
                q.dma_start(tl[:], gb_d[:, t * 8:(t + 1) * 8, m0:m0 + mw])
            gbt[(s, t)] = tl

        issue_gb(0, 0, split=True)

        # Resident W: 64 individual [128, 2, OUT_W] tiles on the sync queue.
        wres = [res.tile([128, 2, OUT_W], FP8, name=f"w{pr}", tag=f"w{pr}")
                for pr in range(NPAIR)]
        for pr in range(8):
            nc.sync.dma_start(wres[pr][:], w_d[pr])
        issue_gb(0, 1)

        carry_z: dict = {}

        def gen_z(s, pr, mcs):
            gi = pr // 2
            xb0 = (pr % 2) * 2
            m0 = mcs[0] * 128
            mw = len(mcs) * 128
            gsl = gbt[(s, gi // 8)][:, gi % 8, :].unsqueeze(1).broadcast_to(
                [128, 2, mw])
            xsl = xt4[:, xb0:xb0 + 2, m0:m0 + mw]
            zt = zp.tile([128, 2, mw], FP8, name=f"zt{s}_{pr}", tag="zt")
            m = pr % 16
            if m in _CAST_DVE or (s == 0 and pr < 4):
                # Direct fp8-out multiply (1x DVE): used for the DVE share
                # and at startup, where an ACT cast would pull the first
                # matmul behind a 1.3us act-table load.
                nc.vector.tensor_mul(zt[:], xsl, gsl)
                return zt
            zb = zbp.tile([128, 2, mw], BF16, name=f"zb{s}_{pr}", tag="zb")
            nc.vector.tensor_mul(zb[:], xsl, gsl)
            if m in _CAST_POOL:
                nc.gpsimd.tensor_copy(zt[:], zb[:])
            else:
                nc.scalar.activation(zt[:], zb[:],
                                     mybir.ActivationFunctionType.Copy)
            return zt

        def pair_matmuls(zt, pr, mcs, pmu, psg, ppi, chunks=None):
            first = pr == 0
            last = pr == NPAIR - 1
            for j, mc in (chunks if chunks is not None else enumerate(mcs)):
                lhs = zt[:, :, j * 128:(j + 1) * 128]
                if last:
                    # sigma first so its drain chain starts earliest
                    nc.tensor.matmul(psg[mc][:], lhs,
                                     wres[pr][:, :, C + CD:],
                                     start=False, stop=True, perf_mode=DR)
                    nc.tensor.matmul(pmu[mc][:], lhs,
                                     wres[pr][:, :, C:C + CD],
                                     start=False, stop=True, perf_mode=DR)
                    nc.tensor.matmul(ppi[:, j * 16:(j + 1) * 16], lhs,
                                     wres[pr][:, :, 0:C], start=False,
                                     stop=True, perf_mode=DR,
                                     skip_group_check=True)
                else:
                    nc.tensor.matmul(pmu[mc][:], lhs,
                                     wres[pr][:, :, C:C + CD],
                                     start=first, stop=False, perf_mode=DR)
                    nc.tensor.matmul(psg[mc][:], lhs,
                                     wres[pr][:, :, C + CD:],
                                     start=first, stop=False, perf_mode=DR)
                    nc.tensor.matmul(ppi[:, j * 16:(j + 1) * 16], lhs,
                                     wres[pr][:, :, 0:C],
                                     start=(first and j == 0), stop=False,
                                     perf_mode=DR, skip_group_check=True)

        for s, mcs in enumerate(SWEEPS):
            ppi = pp.tile([128, 16 * len(mcs)], F32, name=f"ppi{s}",
                          tag="ppi", bufs=2)
            pmu, psg = {}, {}
            for j, mc in enumerate(mcs):
                pmu[mc] = pp.tile([128, CD], F32, name=f"pmu{s}_{j}",
                                  tag="pmu", bufs=3)
                psg[mc] = pp.tile([128, CD], F32, name=f"psg{s}_{j}",
                                  tag="psg", bufs=3)

            # Carried pairs from the previous sweep run chunk-major, so the
            # PE restarts on chunk 0 the moment its drain adds retire.
            start_pr = 0
            if s > 0:
                nc_s = NCARRY[s]
                for j, mc in enumerate(mcs):
                    for pr in range(nc_s):
                        pair_matmuls(carry_z[(s, pr)], pr, mcs, pmu, psg,
                                     ppi, chunks=[(j, mc)])
                for pr in range(nc_s):
                    del carry_z[(s, pr)]
                start_pr = nc_s

            for pr in range(start_pr, NPAIR):
                if s == 0:
                    # Keep the resident-W queue ~8 pairs ahead of the PE.
                    if pr + 8 < NPAIR:
                        nc.sync.dma_start(wres[pr + 8][:], w_d[pr + 8])
                    if pr == 8:
                        issue_gb(0, 2)
                    elif pr == 24:
                        issue_gb(0, 3)
                    elif pr == 30:
                        # x^T columns for sweep 2 only; sweep 3's load waits
                        # until the W stream is done paying for sweep 1.
                        nc.sync.dma_start(xt4[:, 0:2, 384:768],
                                          xt_d[:, 0:2, 384:768])
                    elif pr == 34:
                        nc.sync.dma_start(xt4[:, 2:4, 384:768],
                                          xt_d[:, 2:4, 384:768])
                else:
                    if pr == start_pr + 2:
                        issue_gb(s, 2)
                    elif pr == start_pr + 10:
                        issue_gb(s, 3)
                    if s == 1 and pr == 14:
                        nc.sync.dma_start(xt4[:, 0:2, 768:],
                                          xt_d[:, 0:2, 768:])
                    elif s == 1 and pr == 16:
                        nc.sync.dma_start(xt4[:, 2:4, 768:],
                                          xt_d[:, 2:4, 768:])
                if pr == (56 if s == 0 else 30):
                    # One bias tile per sweep; after the last W issue in
                    # sweep 1 so its transfer never stalls the W stream.
                    bt = bp.tile([128, len(mcs), OUT_W], F32, name=f"bt{s}",
                                 tag="bt")
                    nc.sync.dma_start(bt[:],
                                      bias_d[:, mcs[0]:mcs[0] + len(mcs), :])
                if s + 1 < len(SWEEPS):
                    q_pref = nc.gpsimd if s == 0 else nc.sync
                    if pr == 38:
                        # During sweep 0 the sync queue is still streaming W.
                        issue_gb(s + 1, 0, q=q_pref)
                    c0 = NPAIR - 2 * NCARRY[s + 1]
                    if pr >= c0 and (pr - c0) % 2 == 0:
                        cpr = (pr - c0) // 2
                        carry_z[(s + 1, cpr)] = gen_z(s + 1, cpr,
                                                      SWEEPS[s + 1])
                zt = gen_z(s, pr, mcs)
                pair_matmuls(zt, pr, mcs, pmu, psg, ppi)

            # The next sweep's second gate tile: issued at the boundary, the
            # transfer rides the DMA lull between sweeps instead of adding
            # 2.2us to the W-paced stream (needed at next-sweep pair 16).
            if s + 1 < len(SWEEPS):
                issue_gb(s + 1, 1, q=nc.gpsimd if s == 0 else nc.sync)

            # Drain. softplus(v) = ln(exp(v) + 1); the reference's +1e-7 is
            # dropped (5e-7 relative effect, far below fp8 noise). The Exp
            # and Ln over all chunks are each ONE wide ACT op: the scheduler
            # cannot interleave them (each interleave costs a 1.3us act-
            # table reload), and the tail shrinks to add->Exp->Ln->store.
            # In the final sweep Ln splits per chunk (slices of one tile, so
            # no buffer pressure can force an Exp/Ln interleave) and each
            # chunk's scale store dispatches right after its Ln.
            nmc_s = len(mcs)
            last_sweep = s == len(SWEEPS) - 1
            ots = {}
            eiT = op.tile([128, nmc_s * CD], F32, name=f"ei{s}", tag="ei",
                          bufs=1)
            for j, mc in enumerate(mcs):
                # Per chunk: ei-add (feeds ACT) then mu-add; together they
                # free this chunk's psum slots for the next sweep. In the
                # final sweep nothing waits on the psum slots, so all
                # ei-adds go first and the Exp->Ln->store tail starts ~1.2us
                # earlier.
                nc.vector.tensor_add(eiT[:, j * CD:(j + 1) * CD], psg[mc][:],
                                     bt[:, j, C + CD:])
                if not last_sweep:
                    ot = op.tile([128, C + CD], F32, name=f"ot{s}_{j}",
                                 tag="ot")
                    nc.vector.tensor_add(ot[:, C:C + CD], pmu[mc][:],
                                         bt[:, j, C:C + CD])
                    ots[mc] = ot
            if last_sweep:
                for j, mc in enumerate(mcs):
                    ot = op.tile([128, C + CD], F32, name=f"ot{s}_{j}",
                                 tag="ot")
                    nc.vector.tensor_add(ot[:, C:C + CD], pmu[mc][:],
                                         bt[:, j, C:C + CD])
                    ots[mc] = ot
            # Exp intermediate in bf16: halves its SBUF and the 0.4% bf16
            # rounding adds ~2.5e-3 to the scale section, inside the budget.
            etT = op.tile([128, nmc_s * CD], BF16, name=f"et{s}", tag="et",
                          bufs=1)
            nc.scalar.activation(etT[:], eiT[:],
                                 mybir.ActivationFunctionType.Exp)
            for j, mc in enumerate(mcs):
                ot = ots[mc]
                nc.vector.tensor_add(ot[:, 0:C], ppi[:, j * 16:(j + 1) * 16],
                                     bt[:, j, 0:C])
                # Dispatch on the sync queue (idle once W is resident): on
                # the ACT queue this store would park at the queue head
                # waiting for the DVE pi-add and block the Ln behind it;
                # anything queued later on sync has tens of us of slack.
                nc.sync.dma_start(out_d[:, mc, 0:C + CD], ot[:])
            lnT = op.tile([128, nmc_s * CD], F32, name=f"ln{s}", tag="ln",
                          bufs=1)
            if last_sweep:
                for j, mc in enumerate(mcs):
                    nc.scalar.activation(lnT[:, j * CD:(j + 1) * CD],
                                         etT[:, j * CD:(j + 1) * CD],
                                         mybir.ActivationFunctionType.Ln,
                                         bias=1.0)
                    nc.scalar.dma_start(out_d[:, mc, C + CD:],
                                        lnT[:, j * CD:(j + 1) * CD])
            else:
                nc.scalar.activation(lnT[:], etT[:],
                                     mybir.ActivationFunctionType.Ln,
                                     bias=1.0)
                for j, mc in enumerate(mcs):
                    nc.scalar.dma_start(out_d[:, mc, C + CD:],
                                        lnT[:, j * CD:(j + 1) * CD])

    nc.compile()
    _cache["nc"] = nc
    return nc


def _prep_shared(W_mu, b_mu, W_sigma, b_sigma, W_pi, b_pi):
    fp8 = ml_dtypes.float8_e4m3
    # Column order matches the reference output: [logits | loc | scale].
    w_cat = np.concatenate([W_pi, W_mu, W_sigma], axis=-1)      # [G, I, 1040]
    # K-tile pairs for DoubleRow: [pair, partition, 2, out] where
    # w_np[pr, p, i, :] = W row k = (2*pr+i)*128 + p.
    w_np = np.ascontiguousarray(
        w_cat.reshape(NPAIR, 2, 128, OUT_W).transpose(0, 2, 1, 3)
        .astype(fp8))
    b_cat = np.concatenate([b_pi, b_mu, b_sigma],
                           axis=-1).astype(np.float32)          # [G, 1040]
    return w_np, b_cat


def _core_inputs(x, g, w_np, b_cat, c):
    bf16 = ml_dtypes.bfloat16
    xs = x[c * BLOC:(c + 1) * BLOC]
    gs = g[c * BLOC:(c + 1) * BLOC]
    # x^T packed [partition, i-block, sample]: xt[p, ib, b] = x[b, ib*128+p]
    xT = np.ascontiguousarray(
        xs.T.astype(bf16).reshape(4, 128, BLOC).transpose(1, 0, 2))
    # gates broadcast across partitions: gb[p, g, b] = g[b, g]
    gT = gs.T.astype(bf16)                                      # [32, 1024]
    gb = np.ascontiguousarray(
        np.broadcast_to(gT[None, :, :], (128, G, BLOC)))        # [128,32,1024]
    # bias packed [partition, chunk, out]: bias[p, mc, o] = (g @ b)[mc*128+p, o]
    bias = np.ascontiguousarray(
        (gs.astype(np.float32) @ b_cat).reshape(NMC, 128, OUT_W)
        .transpose(1, 0, 2))
    return {"xt": xT, "gb": gb, "w": w_np, "bias": bias}


def kernel(x, g, W_mu, b_mu, W_sigma, b_sigma, W_pi, b_pi):
    nc = _build_program()
    w_np, b_cat = _prep_shared(W_mu, b_mu, W_sigma, b_sigma, W_pi, b_pi)
    in_maps = [_core_inputs(x, g, w_np, b_cat, c) for c in range(NCORES)]
    res = run_bass_kernel_spmd(nc, in_maps, core_ids=list(range(NCORES)))
    out = np.concatenate(
        [res.results[c]["out"].transpose(1, 0, 2).reshape(BLOC, OUT_W)
         for c in range(NCORES)], axis=0)
    return np.ascontiguousarray(out.astype(np.float32))
